# revision 20
# baseline (speedup 1.0000x reference)
"""AttnBlock (GroupNorm + single-head spatial attention + residual) on 8
Trainium2 NeuronCores.

Sharding: data-parallel over B (4 batches) x 2-way query-sequence parallel =
8 shards. Each core gets the full x[b] (rolled so its query half is the
first 2048 spatial positions), computes GroupNorm + attention for its 2048
queries + residual, and writes a [512, 2048] slice of the output.

Algebraic restructure (vs the q/k/v/out-proj formulation): softmax is
invariant to per-query score offsets and normalization commutes with Wo, so
    scores[q,s] = (M^T h_q)^T h_s   with M = Wq^T Wk
    out[:,q]    = (sum_s e[s,q] * (Wo Wv) h_s) / Z[q] + (Wo bv + bo) + x[:,q]
Precomputing M and Wov = Wo Wv host-side (512x512 each) removes the Q/K
projections and the output projection from the device: only q' = M^T h_q
(queries only -- half of S!) and v' = Wov h remain, and the attention
accumulator po in PSUM f32 is normalized and written out directly.

Compute layout (per core, C=512, S=4096, Sq=2048):
  x        [c, s]   4 chunks of [128, 4096] f16 (channels on partitions;
                    doubles as the residual -- f16 rounding of x adds
                    ~5e-4 relative error, far under the 2e-2 gate)
  h = GN(x)         fp8, block-interleaved [p, u, s-block, j, col] so every
                    matmul can run fp8 DoubleRow (pair dim j at 512B stride)
  q' = M^T h_q      same interleaved fp8 layout, queries only (4 s-blocks)
  vT' = h^T Wov^T   32 tiles of [128, 512] fp8 (spatial on partitions)
  scoresT[s,q] = h_s^T q'_q  per (128-key-tile x 512-query-block) in PSUM --
                 fp8 DoubleRow, 2 instructions per tile (was 4 fp16), with
                 the raw interleaved h8 as the key-side operand.
                 exp()'d on ScalarE into fp8 (x 2^-2 so it can't overflow).
                 Key loop software-pipelined (scores/exp of tile t+1 before
                 the AV matmuls of tile t) so the PE never waits on exp.
  po[c,q] += vT'^T e   accumulated over all 32 key tiles in 4 PSUM banks
  Z[q]    += ones^T e  (full 128-wide fp8 ones lhsT: fast weight load AND
                 broadcasts Z across partitions for free)
  out = x[:, :2048] + po * (1/Z) + bo'   -- the 2^-4 exp shift cancels
                 between po and Z. 1/Z = Exp(-Ln(Z)) on ScalarE (same ACT
                 table set, ~2.4x cheaper than DVE reciprocal); the po*rz
                 muls and residual adds split across DVE and GpSimd so the
                 kernel tail is short. Output written f16 (host casts).

All matmuls are fp8e4m3 with perf_mode=DoubleRow: two 128-rows of
contraction per PE pass, so each [256-contraction x 512-col] instruction
costs ~216ns -- the PE runs at its fp8 peak throughout. fp8 noise on the
scores side averages across the 512-wide contraction and the softmax; on
the value side across 4096 keys. numpy-mirror-predicted end-to-end error
~3e-3 rms of absmax; measured max ~1.3e-2 (gate 2e-2).

GroupNorm, fully per-chunk (each 128-channel chunk holds 8 whole groups of
16 channels, so chunk i's h8 is written as soon as chunk i's x lands and
the projections start right after the last chunk arrives). The elementwise
stats cost is spread over every engine so no single engine serializes the
load phase: sum-over-s runs on the otherwise-idle PE (indicator matmuls
against raw x, columns folded on DVE), sumsq splits ScalarE (Square+accum)
/ Pool (stt+accum), the h8 writes split ScalarE/DVE, and the rstd chain
(exp(-0.5*ln(var+eps)) + one Newton step, all in the preloaded ACT table
set) stays on ScalarE/DVE tiny ops.

DMA discipline: the engines service all enqueued transfers round-robin
concurrently, so x chunks are stacked FIFO across the HW queues
(chunk-major) to finish in order; weights queue behind x; small constants
ride the SW queues.
"""
import numpy as np

import bass_rust
import concourse.bass as bass
import concourse.tile as tile
from concourse import mybir
from concourse.bass_utils import run_bass_kernel_spmd

F32 = mybir.dt.float32
F16 = mybir.dt.float16
F8 = mybir.dt.float8e4
AF = mybir.ActivationFunctionType
ALU = mybir.AluOpType

B, C, H, W = 4, 512, 64, 64
S = H * W            # 4096 spatial positions (keys)
SQ = S // 2          # 2048 queries per core
CC = C // 128        # 4 channel chunks
ST = S // 128        # 32 key tiles
SB = S // 512        # 8 column blocks
QB = SQ // 512       # 4 query blocks
NG = 32              # groups
GS = C // NG         # 16 channels per group
NGC = NG // CC       # 8 groups per 128-channel chunk
EPS = 1e-6
SCALE = 1.0 / float(np.sqrt(C))
# exp() pre-shift: e*2^-4 fits fp8e4m3 (max finite 240). Real max score is
# ~7.3; the overflow threshold ln(240)+4ln2 = 8.25 leaves ~1.0 of headroom
# (a -2ln2 shift measurably overflowed one query).
E8SHIFT = -4.0 * float(np.log(2.0))
DR = mybir.MatmulPerfMode.DoubleRow


def _split_excess_waits(nc, max_waits=1):
    """walrus in this toolchain rejects instructions with >1 sync-wait.
    Hoist excess waits onto same-engine NOPs placed just before the
    instruction (engine streams are in-order, so this is equivalent)."""
    for f in nc.m.functions:
        for bb in f.blocks:
            out = []
            for inst in bb.instructions:
                si = inst.sync_info
                if si is not None and len(si.on_wait) > max_waits:
                    waits = list(si.on_wait)
                    plain = [w for w in waits if w.wait_reg is None]
                    special = [w for w in waits if w.wait_reg is not None]
                    n_keep = max(0, max_waits - len(special))
                    hoist = plain[: len(plain) - n_keep] if n_keep < len(plain) else []
                    keep = plain[len(hoist):] + special
                    if len(keep) > max_waits:
                        out.append(inst)
                        continue
                    for j, w in enumerate(hoist):
                        nop = mybir.InstNoOp(name=f"{inst.name}-wsplit{j}")
                        nop.engine = inst.engine
                        nop.sync_info = bass_rust.SyncInfo(on_wait=[w], on_update=[])
                        out.append(nop)
                    inst.sync_info = bass_rust.SyncInfo(
                        on_wait=keep, on_update=list(si.on_update))
                out.append(inst)
            bb.instructions = out


def _build(with_bo=False):
    nc = bass.Bass(trn_type="TRN2")

    x_d = nc.dram_tensor("x16", [C, S], F16, kind="ExternalInput")
    w8_d = {n: nc.dram_tensor(n, [128, 2, 2, C], F8, kind="ExternalInput")
            for n in ("w8m", "w8ov")}
    bo_d = nc.dram_tensor("boc", [128, CC], F32, kind="ExternalInput")
    ga_d = nc.dram_tensor("gammac", [128, CC], F32, kind="ExternalInput")
    be_d = nc.dram_tensor("betac", [128, CC], F32, kind="ExternalInput")
    ind16_d = nc.dram_tensor("ind16", [128, NGC], F16, kind="ExternalInput")
    indT_d = nc.dram_tensor("indT8", [NGC, 128], F32, kind="ExternalInput")
    out_d = nc.dram_tensor("out", [CC, 128, SQ], F16, kind="ExternalOutput")

    with tile.TileContext(nc) as tc:
        from contextlib import ExitStack
        with ExitStack() as stack:
            const = stack.enter_context(tc.tile_pool(name="const", bufs=1))
            work = stack.enter_context(tc.tile_pool(name="work", bufs=3))
            p_x = stack.enter_context(tc.tile_pool(name="p_x", bufs=1))
            p_h = stack.enter_context(tc.tile_pool(name="p_h", bufs=1))

            w8_sb = {}
            for n in ("w8m", "w8ov"):
                w8_sb[n] = const.tile([128, 2, 2, C], F8, name=f"{n}_sb")

            def emit_weight_dmas():
                # behind x on the HW queues: x keeps full HBM bandwidth and
                # the weights still land well before the projections run
                for n in ("w8m", "w8ov"):
                    nc.sync.dma_start(out=w8_sb[n][:], in_=w8_d[n][:, :, :, :])

            bo_sb = const.tile([128, CC], F32, name="bo_sb")
            nc.gpsimd.dma_start(out=bo_sb[:], in_=bo_d[:, :])
            ga_sb = const.tile([128, CC], F32, name="ga_sb")
            nc.gpsimd.dma_start(out=ga_sb[:], in_=ga_d[:, :])
            be_sb = const.tile([128, CC], F32, name="be_sb")
            nc.gpsimd.dma_start(out=be_sb[:], in_=be_d[:, :])
            ind16_sb = const.tile([128, NGC], F16, name="ind16_sb")
            nc.gpsimd.dma_start(out=ind16_sb[:], in_=ind16_d[:, :])
            indT_sb = const.tile([NGC, 128], F32, name="indT_sb")
            nc.gpsimd.dma_start(out=indT_sb[:], in_=indT_d[:, :])

            # full-width ones pair-tile for the DoubleRow Z matmul: its
            # PSUM output is Z broadcast across all 128 partitions for free
            ones8 = const.tile([128, 2, 128], F8, name="ones8")
            nc.vector.memset(ones8[:], 1.0)
            e8b_sb = const.tile([128, 1], F32, name="e8b_sb")
            nc.vector.memset(e8b_sb[:], E8SHIFT)
            eps_sb = const.tile([NGC, 1], F32, name="eps_sb")
            nc.vector.memset(eps_sb[:], EPS)

            h8 = p_h.tile([128, 2, SB, 2, 512], F8, name="h8")
            q8t = p_h.tile([128, 2, QB, 2, 512], F8, name="q8t")
            vT8 = p_h.tile([128, ST, C], F8, name="vT8")
            xc = p_x.tile([128, CC, S], F16, name="xc")

            # warm the ScalarE natural_log_exp table set while the input DMAs
            # are still in flight (the set load is ~2.7us and all ACT
            # functions used below -- Ln/Exp/Identity/Copy -- live in it)
            warm = work.tile([1, 2], F32, name="warm", tag="warm")
            nc.vector.memset(warm[:], 0.0)
            nc.scalar.activation(warm[:, 1:2], warm[:, 0:1], AF.Exp)

            # =========== Phase 1: load x + GroupNorm (per chunk) ===========
            # Each 128-channel chunk holds 8 complete groups, so its stats,
            # rstd chain and h8 write depend only on that chunk's x -- chunk
            # i's h8 lands ~1.5us after its x DMA and the projections start
            # as soon as chunk 3 is in (~14us) instead of after a full-C
            # GroupNorm (~31us).
            with tc.tile_pool(name="p_gn", bufs=1) as p_gn, \
                 tc.tile_pool(name="ps_gn", bufs=2, space="PSUM") as ps_gn:
                # full-chunk DMAs: 8KB contiguous per partition line (the
                # DMA engines are packet-overhead-bound, so line length is
                # what sets the achievable HBM rate)
                for i in range(CC):
                    nc.sync.dma_start(out=xc[:, i, :],
                                      in_=x_d[i * 128:(i + 1) * 128, :])
                emit_weight_dmas()

                for i in range(CC):
                    # Group sums of x AND x^2 via PE indicator matmuls --
                    # the PE is idle during the x load, and this removes
                    # ~30us of elementwise reduction from ScalarE/DVE.
                    # Pool (otherwise unusable: no pointer scalars, no
                    # free-axis reduce, no PSUM) computes the raw x*x
                    # product in halves; the PE reduces both stats over the
                    # chunk's channels per 512-column block, accumulating
                    # in PSUM [8, 512]; DVE folds the columns. ind16 holds
                    # plain 1.0 (the 1/(GS*S)=2^-16 scale would be an f16
                    # subnormal) -- the scale is applied in the fold.
                    psg_x = ps_gn.tile([NGC, 512], F32, name="psg_x",
                                       tag="psgx", bufs=2)
                    for k in range(8):
                        scols = slice(k * 512, (k + 1) * 512)
                        nc.tensor.matmul(psg_x[:], ind16_sb[:],
                                         xc[:, i, scols],
                                         start=(k == 0), stop=(k == 7))
                    psg_sq = ps_gn.tile([NGC, 512], F32, name="psg_sq",
                                        tag="psgsq", bufs=2)
                    for hh in range(2):
                        hcols = slice(hh * 2048, (hh + 1) * 2048)
                        sq = p_gn.tile([128, 2048], F16, name="sq",
                                       tag=f"sq{hh}", bufs=2)
                        nc.gpsimd.tensor_tensor(sq[:], xc[:, i, hcols],
                                                xc[:, i, hcols], ALU.mult)
                        for k in range(4):
                            scols = slice(k * 512, (k + 1) * 512)
                            nc.tensor.matmul(psg_sq[:], ind16_sb[:],
                                             sq[:, scols],
                                             start=(hh == 0 and k == 0),
                                             stop=(hh == 1 and k == 3))

                    # fold columns + apply the 1/(GS*S) scale:
                    # gstat = [mean, E[x^2]] (DVE: the folds read PSUM)
                    gstat = work.tile([NGC, 2], F32, name="gstat",
                                      tag="gstat", bufs=2)
                    musum = work.tile([NGC, 2], F32, name="musum",
                                      tag="musum", bufs=2)
                    nc.vector.tensor_reduce(out=musum[:, 0:1], in_=psg_x[:],
                                            axis=mybir.AxisListType.X,
                                            op=ALU.add)
                    nc.vector.tensor_reduce(out=musum[:, 1:2], in_=psg_sq[:],
                                            axis=mybir.AxisListType.X,
                                            op=ALU.add)
                    nc.vector.tensor_scalar_mul(gstat[:], musum[:],
                                                1.0 / (GS * S))

                    # rstd_g = (var+eps)^-0.5 via exp(-0.5*ln(var+eps)) --
                    # Ln/Exp share the loaded table set -- plus one Newton
                    # step for full fp32 accuracy
                    nve = work.tile([NGC, 1], F32, name="nve", tag="nve",
                                    bufs=2)
                    nc.vector.scalar_tensor_tensor(
                        out=nve[:], in0=gstat[:, 0:1], scalar=gstat[:, 0:1],
                        in1=gstat[:, 1:2], op0=ALU.mult, op1=ALU.subtract)
                    lnv = work.tile([NGC, 1], F32, name="lnv", tag="lnv",
                                    bufs=2)
                    nc.scalar.activation(lnv[:], nve[:], AF.Ln, scale=-1.0,
                                         bias=eps_sb[:])
                    r0 = work.tile([NGC, 1], F32, name="r0", tag="r0", bufs=2)
                    nc.scalar.activation(r0[:], lnv[:], AF.Exp, scale=-0.5)
                    ve = work.tile([NGC, 1], F32, name="ve", tag="ve", bufs=2)
                    nc.scalar.activation(ve[:], nve[:], AF.Identity,
                                         scale=-1.0, bias=eps_sb[:])
                    r0sq = work.tile([NGC, 1], F32, name="r0sq", tag="r0sq",
                                     bufs=2)
                    nc.vector.tensor_mul(r0sq[:], r0[:], r0[:])
                    t2 = work.tile([NGC, 1], F32, name="t2", tag="t2", bufs=2)
                    nc.vector.tensor_mul(t2[:], ve[:], r0sq[:])
                    t3 = work.tile([NGC, 1], F32, name="t3", tag="t3", bufs=2)
                    nc.vector.tensor_scalar(out=t3[:], in0=t2[:],
                                            scalar1=-0.5, scalar2=1.5,
                                            op0=ALU.mult, op1=ALU.add)
                    gv = work.tile([NGC, 2], F32, name="gv", tag="gv", bufs=2)
                    nc.vector.tensor_copy(gv[:, 0:1], gstat[:, 0:1])
                    nc.vector.tensor_mul(gv[:, 1:2], r0[:], t3[:])

                    # broadcast group stats back to the chunk's channels;
                    # sc = rstd*gamma, bi = mean*sc - beta (DVE quarters),
                    # bn = -bi (ScalarE quarters)
                    psb = ps_gn.tile([128, 2], F32, name="psb", tag="psb",
                                     bufs=2)
                    nc.tensor.matmul(psb[:], indT_sb[:], gv[:],
                                     start=True, stop=True)
                    sc_c = work.tile([128, 1], F32, name="sc_c", tag="gn_sc",
                                     bufs=2)
                    nc.vector.tensor_mul(sc_c[:], psb[:, 1:2],
                                         ga_sb[:, i:i + 1])
                    bi_c = work.tile([128, 1], F32, name="bi_c", tag="gn_bi",
                                     bufs=2)
                    nc.vector.scalar_tensor_tensor(
                        out=bi_c[:], in0=psb[:, 0:1], scalar=sc_c[:],
                        in1=be_sb[:, i:i + 1], op0=ALU.mult, op1=ALU.subtract)
                    bn_c = work.tile([128, 1], F32, name="bn_c", tag="gn_bn",
                                     bufs=2)
                    nc.vector.tensor_scalar_mul(bn_c[:], bi_c[:], -1.0)

                    # h = x*sc - bi, cast to fp8, written per quarter
                    # alternating ScalarE/DVE so the two engines pipeline
                    for qq in range(4):
                        qcols = slice(qq * 1024, (qq + 1) * 1024)
                        hslc = h8[:, i // 2, 2 * qq:2 * qq + 2, i % 2, :]
                        if qq % 2 == 0:
                            nc.scalar.activation(hslc, xc[:, i, qcols],
                                                 AF.Identity,
                                                 bias=bn_c[:], scale=sc_c[:])
                        else:
                            nc.vector.tensor_scalar(
                                out=hslc, in0=xc[:, i, qcols],
                                scalar1=sc_c[:], scalar2=bi_c[:],
                                op0=ALU.mult, op1=ALU.subtract)

            # =========== Phase 2: q'/v' projections ===========
            with tc.tile_pool(name="ps_proj", bufs=3, space="PSUM") as ps_p:
                # q' = M^T h_q (queries only), stored in the interleaved fp8
                # layout (out-chunk oc -> (u=oc//2, j=oc%2)) so the scores
                # matmul runs DoubleRow. Evacuations alternate ScalarE/DVE.
                for oc in range(CC):
                    for qb in range(QB):
                        pt = ps_p.tile([128, 512], F32, name="pt", tag="pp")
                        for u in range(2):
                            nc.tensor.matmul(
                                pt[:],
                                w8_sb["w8m"][:, u, :, oc * 128:(oc + 1) * 128],
                                h8[:, u, qb, :, :],
                                start=(u == 0), stop=(u == 1), perf_mode=DR)
                        dst = q8t[:, oc // 2, qb, oc % 2, :]
                        if (oc * QB + qb) % 2 == 0:
                            nc.scalar.copy(dst, pt[:])
                        else:
                            nc.vector.tensor_copy(dst, pt[:])
                # vT'[s, c] = h[:, s]^T Wov^T  (spatial on partitions)
                for st in range(ST):
                    pt = ps_p.tile([128, 512], F32, name="pt", tag="pp")
                    ccol = slice((st % 4) * 128, (st % 4) * 128 + 128)
                    for u in range(2):
                        nc.tensor.matmul(pt[:], h8[:, u, st // 4, :, ccol],
                                         w8_sb["w8ov"][:, u, :, :],
                                         start=(u == 0), stop=(u == 1),
                                         perf_mode=DR)
                    if st % 2 == 0:
                        nc.scalar.copy(vT8[:, st, :], pt[:])
                    else:
                        nc.vector.tensor_copy(vT8[:, st, :], pt[:])

            # =========== Phase 3: attention ===========
            with tc.tile_pool(name="ps_po", bufs=4, space="PSUM") as ps_po, \
                 tc.tile_pool(name="ps_z", bufs=1, space="PSUM") as ps_z, \
                 tc.tile_pool(name="ps_s", bufs=3, space="PSUM") as ps_s:

                NP = ST // 2   # key-tile pairs (fp8 DoubleRow packs 2)

                def emit_scores_pair(qb, t):
                    e8p = work.tile([128, 2, 512], F8, name="e8p",
                                    tag="e8p", bufs=3)
                    for j in range(2):
                        st = 2 * t + j
                        pscore = ps_s.tile([128, 512], F32, name="pscore",
                                           tag="msum")
                        sc128 = slice((st % 4) * 128, (st % 4) * 128 + 128)
                        for u in range(2):
                            nc.tensor.matmul(
                                pscore[:], h8[:, u, st // 4, :, sc128],
                                q8t[:, u, qb, :, :],
                                start=(u == 0), stop=(u == 1), perf_mode=DR)
                        # e' = exp(score/sqrt(C)) * 2^-2 so fp8e4m3 never
                        # overflows; the shift cancels against Z in the
                        # final normalization
                        nc.scalar.activation(e8p[:, j, :], pscore[:], AF.Exp,
                                             scale=SCALE, bias=e8b_sb[:])
                    return e8p

                def emit_av(po, pz, t, e8p):
                    for cc2 in range(CC):
                        nc.tensor.matmul(
                            po[cc2][:],
                            vT8[:, 2 * t:2 * t + 2, cc2 * 128:(cc2 + 1) * 128],
                            e8p[:],
                            start=(t == 0), stop=(t == NP - 1), perf_mode=DR)
                    nc.tensor.matmul(pz[:], ones8[:], e8p[:],
                                     start=(t == 0), stop=(t == NP - 1),
                                     perf_mode=DR)

                for qb in range(QB):
                    po = [ps_po.tile([128, 512], F32, name="po", tag="po")
                          for _ in range(CC)]
                    pz = ps_z.tile([128, 512], F32, name="pz", tag="pz")
                    # software-pipelined: scores/exp for pair t+1 are
                    # issued before the AV matmuls of pair t, so the PE
                    # never waits on the ScalarE exp.
                    e_prev = emit_scores_pair(qb, 0)
                    for t in range(1, NP):
                        e_cur = emit_scores_pair(qb, t)
                        emit_av(po, pz, t - 1, e_prev)
                        e_prev = e_cur
                    emit_av(po, pz, NP - 1, e_prev)
                    # normalize + bias + residual + writeout. 1/Z =
                    # exp(-ln(Z)) on ScalarE (cheap, same table set); the
                    # po*rz muls and residual adds split DVE/GpSimd so the
                    # PSUM banks recycle fast and the last block's tail is
                    # short. The 2^-2 exp shift cancels between po and Z.
                    qcols = slice(qb * 512, (qb + 1) * 512)
                    zln = work.tile([128, 512], F32, name="zln", tag="zln",
                                    bufs=2)
                    nc.scalar.activation(zln[:], pz[:], AF.Ln)
                    rzb = work.tile([128, 512], F32, name="rzb", tag="rzb",
                                    bufs=2)
                    nc.scalar.activation(rzb[:], zln[:], AF.Exp, scale=-1.0)
                    for oc in range(CC):
                        # GPSIMD can't read PSUM: the po*rz mul stays on
                        # DVE; the residual add (SBUF-only) alternates onto
                        # GpSimd so the DVE tail chain stays short. Pool
                        # rejects pointer-scalar ops, so the (normally
                        # all-zero) bo bias forces the DVE path instead.
                        t32 = work.tile([128, 512], F32, name="t32",
                                        tag=f"t32_{oc % 2}", bufs=2)
                        nc.vector.tensor_mul(t32[:], po[oc][:], rzb[:])
                        o16 = work.tile([128, 512], F16, name="o16",
                                        tag=f"o16_{oc % 2}", bufs=2)
                        if with_bo:
                            nc.vector.scalar_tensor_tensor(
                                out=o16[:], in0=t32[:],
                                scalar=bo_sb[:, oc:oc + 1],
                                in1=xc[:, oc, qcols],
                                op0=ALU.add, op1=ALU.add)
                        else:
                            eng = nc.gpsimd if oc % 2 else nc.vector
                            eng.tensor_add(o16[:], t32[:], xc[:, oc, qcols])
                        nc.sync.dma_start(out=out_d[oc, :, qcols], in_=o16[:])

    _split_excess_waits(nc)
    return nc


_cache = {}


def _get_program(with_bo=False):
    key = ("nc", with_bo)
    if key not in _cache:
        _cache[key] = _build(with_bo)
    return _cache[key]


def kernel(x, gamma, beta, wq, bq, wk, bk, wv, bv, wo, bo, trace=False):
    x = np.asarray(x, dtype=np.float32)
    gamma = np.asarray(gamma, dtype=np.float32)
    beta = np.asarray(beta, dtype=np.float32)
    wq, wk, wv, wo = (np.asarray(a, dtype=np.float32) for a in (wq, wk, wv, wo))
    bq, bk, bv, bo = (np.asarray(a, dtype=np.float32) for a in (bq, bk, bv, bo))
    assert not (np.any(bq) or np.any(bk)), \
        "nonzero bq/bk not supported by the fused-scores fast path"

    bo_eff = wo @ bv + bo
    nc = _get_program(with_bo=bool(np.any(bo_eff)))

    f8np = mybir.dt.np(F8)

    def pack8(w):
        wt = np.ascontiguousarray(w.T.astype(np.float32))
        return np.ascontiguousarray(
            wt.reshape(2, 2, 128, C).transpose(2, 0, 1, 3)).astype(f8np)

    # fold the q/k projections into M (applied to the query side only) and
    # the v/out projections into Wov; bv rides along as a constant output
    # offset (sum_s softmax = 1)
    M_T = wk.T @ wq          # device computes q' = (M_T) h_q = M^T h_q
    Wov = wo @ wv

    shared = {
        "w8m": pack8(M_T), "w8ov": pack8(Wov),
        "boc": np.ascontiguousarray(bo_eff.reshape(CC, 128).T),
        "gammac": np.ascontiguousarray(gamma.reshape(CC, 128).T),
        "betac": np.ascontiguousarray(beta.reshape(CC, 128).T),
    }
    # per-chunk group indicators: every 128-channel chunk holds the same
    # local group pattern (group = p // 16)
    ind16 = np.zeros((128, NGC), np.float16)
    indT8 = np.zeros((NGC, 128), np.float32)
    for p in range(128):
        ind16[p, p // GS] = 1.0   # plain 1.0 (2^-16 scale applied on-device)
        indT8[p // GS, p] = 1.0
    shared["ind16"] = ind16
    shared["indT8"] = indT8

    in_maps = []
    for core in range(8):
        b, half = core // 2, core % 2
        xs = x[b].reshape(C, S)
        if half:
            xin = np.concatenate([xs[:, SQ:], xs[:, :SQ]], axis=1)
        else:
            xin = np.ascontiguousarray(xs)
        in_maps.append({"x16": xin.astype(np.float16), **shared})

    res = run_bass_kernel_spmd(nc, in_maps, core_ids=list(range(8)),
                               trace=trace)
    _cache["last_exec_time_ns"] = res.exec_time_ns

    y = np.empty((B, C, S), np.float32)
    for core in range(8):
        b, half = core // 2, core % 2
        y[b, :, half * SQ:(half + 1) * SQ] = \
            res.results[core]["out"].reshape(C, SQ).astype(np.float32)
    return y.reshape(B, C, H, W)


# revision 21
# speedup vs baseline: 1.0809x; 1.0809x over previous
"""AttnBlock (GroupNorm + single-head spatial attention + residual) on 8
Trainium2 NeuronCores.

Sharding: data-parallel over B (4 batches) x 2-way query-sequence parallel =
8 shards. Each core gets the full x[b] (rolled so its query half is the
first 2048 spatial positions), computes GroupNorm + attention for its 2048
queries + residual, and writes a [512, 2048] slice of the output.

Algebraic restructure (vs the q/k/v/out-proj formulation): softmax is
invariant to per-query score offsets and normalization commutes with Wo, so
    scores[q,s] = (M^T h_q)^T h_s   with M = Wq^T Wk
    out[:,q]    = (sum_s e[s,q] * (Wo Wv) h_s) / Z[q] + (Wo bv + bo) + x[:,q]
Precomputing M and Wov = Wo Wv host-side (512x512 each) removes the Q/K
projections and the output projection from the device: only q' = M^T h_q
(queries only -- half of S!) and v' = Wov h remain, and the attention
accumulator po in PSUM f32 is normalized and written out directly.

Compute layout (per core, C=512, S=4096, Sq=2048):
  x        [c, s]   4 chunks of [128, 4096] f16 (channels on partitions;
                    doubles as the residual -- f16 rounding of x adds
                    ~5e-4 relative error, far under the 2e-2 gate)
  h = GN(x)         fp8, block-interleaved [p, u, s-block, j, col] so every
                    matmul can run fp8 DoubleRow (pair dim j at 512B stride)
  q' = M^T h_q      same interleaved fp8 layout, queries only (4 s-blocks)
  vT' = h^T Wov^T   32 tiles of [128, 512] fp8 (spatial on partitions)
  scoresT[s,q] = h_s^T q'_q  per (128-key-tile x 512-query-block) in PSUM --
                 fp8 DoubleRow, 2 instructions per tile (was 4 fp16), with
                 the raw interleaved h8 as the key-side operand.
                 exp()'d on ScalarE into fp8 (x 2^-2 so it can't overflow).
                 Key loop software-pipelined (scores/exp of tile t+1 before
                 the AV matmuls of tile t) so the PE never waits on exp.
  po[c,q] += vT'^T e   accumulated over all 32 key tiles in 4 PSUM banks
  Z[q]    += ones^T e  (full 128-wide fp8 ones lhsT: fast weight load AND
                 broadcasts Z across partitions for free)
  out = x[:, :2048] + po * (1/Z) + bo'   -- the 2^-4 exp shift cancels
                 between po and Z. 1/Z = Exp(-Ln(Z)) on ScalarE (same ACT
                 table set, ~2.4x cheaper than DVE reciprocal); the po*rz
                 muls and residual adds split across DVE and GpSimd so the
                 kernel tail is short. Output written f16 (host casts).

All matmuls are fp8e4m3 with perf_mode=DoubleRow: two 128-rows of
contraction per PE pass, so each [256-contraction x 512-col] instruction
costs ~216ns -- the PE runs at its fp8 peak throughout. fp8 noise on the
scores side averages across the 512-wide contraction and the softmax; on
the value side across 4096 keys. numpy-mirror-predicted end-to-end error
~3e-3 rms of absmax; measured max ~1.3e-2 (gate 2e-2).

GroupNorm, fully per-chunk (each 128-channel chunk holds 8 whole groups of
16 channels, so chunk i's h8 is written as soon as chunk i's x lands and
the projections start right after the last chunk arrives). The elementwise
stats cost is spread over every engine so no single engine serializes the
load phase: sum-over-s runs on the otherwise-idle PE (indicator matmuls
against raw x, columns folded on DVE), sumsq splits ScalarE (Square+accum)
/ Pool (stt+accum), the h8 writes split ScalarE/DVE, and the rstd chain
(exp(-0.5*ln(var+eps)) + one Newton step, all in the preloaded ACT table
set) stays on ScalarE/DVE tiny ops.

DMA discipline: the engines service all enqueued transfers round-robin
concurrently, so x chunks are stacked FIFO across the HW queues
(chunk-major) to finish in order; weights queue behind x; small constants
ride the SW queues.
"""
import numpy as np

import bass_rust
import concourse.bass as bass
import concourse.tile as tile
from concourse import mybir
from concourse.bass_utils import run_bass_kernel_spmd

F32 = mybir.dt.float32
F16 = mybir.dt.float16
F8 = mybir.dt.float8e4
AF = mybir.ActivationFunctionType
ALU = mybir.AluOpType

B, C, H, W = 4, 512, 64, 64
S = H * W            # 4096 spatial positions (keys)
SQ = S // 2          # 2048 queries per core
CC = C // 128        # 4 channel chunks
ST = S // 128        # 32 key tiles
SB = S // 512        # 8 column blocks
QB = SQ // 512       # 4 query blocks
NG = 32              # groups
GS = C // NG         # 16 channels per group
NGC = NG // CC       # 8 groups per 128-channel chunk
EPS = 1e-6
SCALE = 1.0 / float(np.sqrt(C))
# exp() pre-shift: e*2^-4 fits fp8e4m3 (max finite 240). Real max score is
# ~7.3; the overflow threshold ln(240)+4ln2 = 8.25 leaves ~1.0 of headroom
# (a -2ln2 shift measurably overflowed one query).
E8SHIFT = -4.0 * float(np.log(2.0))
DR = mybir.MatmulPerfMode.DoubleRow


def _split_excess_waits(nc, max_waits=1):
    """walrus in this toolchain rejects instructions with >1 sync-wait.
    Hoist excess waits onto same-engine NOPs placed just before the
    instruction (engine streams are in-order, so this is equivalent)."""
    for f in nc.m.functions:
        for bb in f.blocks:
            out = []
            for inst in bb.instructions:
                si = inst.sync_info
                if si is not None and len(si.on_wait) > max_waits:
                    waits = list(si.on_wait)
                    plain = [w for w in waits if w.wait_reg is None]
                    special = [w for w in waits if w.wait_reg is not None]
                    n_keep = max(0, max_waits - len(special))
                    hoist = plain[: len(plain) - n_keep] if n_keep < len(plain) else []
                    keep = plain[len(hoist):] + special
                    if len(keep) > max_waits:
                        out.append(inst)
                        continue
                    for j, w in enumerate(hoist):
                        nop = mybir.InstNoOp(name=f"{inst.name}-wsplit{j}")
                        nop.engine = inst.engine
                        nop.sync_info = bass_rust.SyncInfo(on_wait=[w], on_update=[])
                        out.append(nop)
                    inst.sync_info = bass_rust.SyncInfo(
                        on_wait=keep, on_update=list(si.on_update))
                out.append(inst)
            bb.instructions = out


def _build(with_bo=False):
    nc = bass.Bass(trn_type="TRN2")

    x_d = nc.dram_tensor("x16", [C, S], F16, kind="ExternalInput")
    w8_d = {n: nc.dram_tensor(n, [128, 2, 2, C], F8, kind="ExternalInput")
            for n in ("w8m", "w8ov")}
    bo_d = nc.dram_tensor("boc", [128, CC], F32, kind="ExternalInput")
    ga_d = nc.dram_tensor("gammac", [128, CC], F32, kind="ExternalInput")
    be_d = nc.dram_tensor("betac", [128, CC], F32, kind="ExternalInput")
    ind_d = nc.dram_tensor("ind8", [128, NGC], F32, kind="ExternalInput")
    ind16_d = nc.dram_tensor("ind16", [128, NGC], F16, kind="ExternalInput")
    indT_d = nc.dram_tensor("indT8", [NGC, 128], F32, kind="ExternalInput")
    out_d = nc.dram_tensor("out", [CC, 128, SQ], F16, kind="ExternalOutput")

    with tile.TileContext(nc) as tc:
        from contextlib import ExitStack
        with ExitStack() as stack:
            const = stack.enter_context(tc.tile_pool(name="const", bufs=1))
            work = stack.enter_context(tc.tile_pool(name="work", bufs=3))
            p_x = stack.enter_context(tc.tile_pool(name="p_x", bufs=1))
            p_h = stack.enter_context(tc.tile_pool(name="p_h", bufs=1))

            w8_sb = {}
            for n in ("w8m", "w8ov"):
                w8_sb[n] = const.tile([128, 2, 2, C], F8, name=f"{n}_sb")

            def emit_weight_dmas():
                # behind x on the HW queues: x keeps full HBM bandwidth and
                # the weights still land well before the projections run
                for n in ("w8m", "w8ov"):
                    nc.sync.dma_start(out=w8_sb[n][:], in_=w8_d[n][:, :, :, :])

            bo_sb = const.tile([128, CC], F32, name="bo_sb")
            nc.gpsimd.dma_start(out=bo_sb[:], in_=bo_d[:, :])
            ga_sb = const.tile([128, CC], F32, name="ga_sb")
            nc.gpsimd.dma_start(out=ga_sb[:], in_=ga_d[:, :])
            be_sb = const.tile([128, CC], F32, name="be_sb")
            nc.gpsimd.dma_start(out=be_sb[:], in_=be_d[:, :])
            ind_sb = const.tile([128, NGC], F32, name="ind_sb")
            nc.gpsimd.dma_start(out=ind_sb[:], in_=ind_d[:, :])
            ind16_sb = const.tile([128, NGC], F16, name="ind16_sb")
            nc.gpsimd.dma_start(out=ind16_sb[:], in_=ind16_d[:, :])
            indT_sb = const.tile([NGC, 128], F32, name="indT_sb")
            nc.gpsimd.dma_start(out=indT_sb[:], in_=indT_d[:, :])

            # full-width ones pair-tile for the DoubleRow Z matmul: its
            # PSUM output is Z broadcast across all 128 partitions for free
            ones8 = const.tile([128, 2, 128], F8, name="ones8")
            nc.vector.memset(ones8[:], 1.0)
            e8b_sb = const.tile([128, 1], F32, name="e8b_sb")
            nc.vector.memset(e8b_sb[:], E8SHIFT)
            eps_sb = const.tile([NGC, 1], F32, name="eps_sb")
            nc.vector.memset(eps_sb[:], EPS)

            h8 = p_h.tile([128, 2, SB, 2, 512], F8, name="h8")
            q8t = p_h.tile([128, 2, QB, 2, 512], F8, name="q8t")
            vT8 = p_h.tile([128, ST, C], F8, name="vT8")
            xc = p_x.tile([128, CC, S], F16, name="xc")

            # warm the ScalarE natural_log_exp table set while the input DMAs
            # are still in flight (the set load is ~2.7us and all ACT
            # functions used below -- Ln/Exp/Identity/Copy -- live in it)
            warm = work.tile([1, 2], F32, name="warm", tag="warm")
            nc.vector.memset(warm[:], 0.0)
            nc.scalar.activation(warm[:, 1:2], warm[:, 0:1], AF.Exp)

            # =========== Phase 1: load x + GroupNorm (per chunk) ===========
            # Each 128-channel chunk holds 8 complete groups, so its stats,
            # rstd chain and h8 write depend only on that chunk's x -- chunk
            # i's h8 lands ~1.5us after its x DMA and the projections start
            # as soon as chunk 3 is in (~14us) instead of after a full-C
            # GroupNorm (~31us).
            with tc.tile_pool(name="p_gn", bufs=1) as p_gn, \
                 tc.tile_pool(name="ps_gn", bufs=2, space="PSUM") as ps_gn:
                # full-chunk DMAs: 8KB contiguous per partition line (the
                # DMA engines are packet-overhead-bound, so line length is
                # what sets the achievable HBM rate)
                for i in range(CC):
                    nc.sync.dma_start(out=xc[:, i, :],
                                      in_=x_d[i * 128:(i + 1) * 128, :])
                emit_weight_dmas()

                for i in range(CC):
                    # Group sums of x AND x^2 via PE indicator matmuls --
                    # the PE is idle during the x load, and this removes
                    # ~30us of elementwise reduction from ScalarE/DVE.
                    # Pool (otherwise unusable: no pointer scalars, no
                    # free-axis reduce, no PSUM) computes the raw x*x
                    # product in halves; the PE reduces both stats over the
                    # chunk's channels per 512-column block, accumulating
                    # in PSUM [8, 512]; DVE folds the columns. ind16 holds
                    # plain 1.0 (the 1/(GS*S)=2^-16 scale would be an f16
                    # subnormal) -- the scale is applied in the fold.
                    psg_x = ps_gn.tile([NGC, 512], F32, name="psg_x",
                                       tag="psgx", bufs=2)
                    for k in range(8):
                        scols = slice(k * 512, (k + 1) * 512)
                        nc.tensor.matmul(psg_x[:], ind16_sb[:],
                                         xc[:, i, scols],
                                         start=(k == 0), stop=(k == 7))
                    # sumsq per channel: ScalarE Square+accum in halves
                    # (the cheapest elementwise engine; Pool stays dark --
                    # lighting a 4th engine here measurably trips the power
                    # throttle and erases the gain)
                    s2q = work.tile([128, 2], F32, name="s2q",
                                    tag="gn_s2q", bufs=2)
                    for hh in range(2):
                        hcols = slice(hh * 2048, (hh + 1) * 2048)
                        sq = p_gn.tile([128, 2048], F8, name="sq",
                                       tag=f"sq{hh}", bufs=2)
                        nc.scalar.activation(sq[:], xc[:, i, hcols],
                                             AF.Square,
                                             accum_out=s2q[:, hh:hh + 1])
                    psg = ps_gn.tile([NGC, 2], F32, name="psg", tag="psg",
                                     bufs=2)
                    nc.tensor.matmul(psg[:], ind_sb[:], s2q[:],
                                     start=True, stop=True)

                    # fold + scale: gstat = [mean, E[x^2]] (DVE reads PSUM)
                    gstat = work.tile([NGC, 2], F32, name="gstat",
                                      tag="gstat", bufs=2)
                    musum = work.tile([NGC, 1], F32, name="musum",
                                      tag="musum", bufs=2)
                    nc.vector.tensor_reduce(out=musum[:], in_=psg_x[:],
                                            axis=mybir.AxisListType.X,
                                            op=ALU.add)
                    nc.vector.tensor_scalar_mul(gstat[:, 0:1], musum[:],
                                                1.0 / (GS * S))
                    nc.vector.tensor_reduce(out=gstat[:, 1:2], in_=psg[:],
                                            axis=mybir.AxisListType.X,
                                            op=ALU.add)

                    # rstd_g = (var+eps)^-0.5 via exp(-0.5*ln(var+eps)) --
                    # Ln/Exp share the loaded table set -- plus one Newton
                    # step for full fp32 accuracy
                    nve = work.tile([NGC, 1], F32, name="nve", tag="nve",
                                    bufs=2)
                    nc.vector.scalar_tensor_tensor(
                        out=nve[:], in0=gstat[:, 0:1], scalar=gstat[:, 0:1],
                        in1=gstat[:, 1:2], op0=ALU.mult, op1=ALU.subtract)
                    lnv = work.tile([NGC, 1], F32, name="lnv", tag="lnv",
                                    bufs=2)
                    nc.scalar.activation(lnv[:], nve[:], AF.Ln, scale=-1.0,
                                         bias=eps_sb[:])
                    r0 = work.tile([NGC, 1], F32, name="r0", tag="r0", bufs=2)
                    nc.scalar.activation(r0[:], lnv[:], AF.Exp, scale=-0.5)
                    ve = work.tile([NGC, 1], F32, name="ve", tag="ve", bufs=2)
                    nc.scalar.activation(ve[:], nve[:], AF.Identity,
                                         scale=-1.0, bias=eps_sb[:])
                    r0sq = work.tile([NGC, 1], F32, name="r0sq", tag="r0sq",
                                     bufs=2)
                    nc.vector.tensor_mul(r0sq[:], r0[:], r0[:])
                    t2 = work.tile([NGC, 1], F32, name="t2", tag="t2", bufs=2)
                    nc.vector.tensor_mul(t2[:], ve[:], r0sq[:])
                    t3 = work.tile([NGC, 1], F32, name="t3", tag="t3", bufs=2)
                    nc.vector.tensor_scalar(out=t3[:], in0=t2[:],
                                            scalar1=-0.5, scalar2=1.5,
                                            op0=ALU.mult, op1=ALU.add)
                    gv = work.tile([NGC, 2], F32, name="gv", tag="gv", bufs=2)
                    nc.vector.tensor_copy(gv[:, 0:1], gstat[:, 0:1])
                    nc.vector.tensor_mul(gv[:, 1:2], r0[:], t3[:])

                    # broadcast group stats back to the chunk's channels;
                    # sc = rstd*gamma, bi = mean*sc - beta (DVE quarters),
                    # bn = -bi (ScalarE quarters)
                    psb = ps_gn.tile([128, 2], F32, name="psb", tag="psb",
                                     bufs=2)
                    nc.tensor.matmul(psb[:], indT_sb[:], gv[:],
                                     start=True, stop=True)
                    sc_c = work.tile([128, 1], F32, name="sc_c", tag="gn_sc",
                                     bufs=2)
                    nc.vector.tensor_mul(sc_c[:], psb[:, 1:2],
                                         ga_sb[:, i:i + 1])
                    bi_c = work.tile([128, 1], F32, name="bi_c", tag="gn_bi",
                                     bufs=2)
                    nc.vector.scalar_tensor_tensor(
                        out=bi_c[:], in0=psb[:, 0:1], scalar=sc_c[:],
                        in1=be_sb[:, i:i + 1], op0=ALU.mult, op1=ALU.subtract)
                    bn_c = work.tile([128, 1], F32, name="bn_c", tag="gn_bn",
                                     bufs=2)
                    nc.vector.tensor_scalar_mul(bn_c[:], bi_c[:], -1.0)

                    # h = x*sc - bi, cast to fp8, written per quarter
                    # alternating ScalarE/DVE so the two engines pipeline
                    for qq in range(4):
                        qcols = slice(qq * 1024, (qq + 1) * 1024)
                        hslc = h8[:, i // 2, 2 * qq:2 * qq + 2, i % 2, :]
                        if qq % 2 == 0:
                            nc.scalar.activation(hslc, xc[:, i, qcols],
                                                 AF.Identity,
                                                 bias=bn_c[:], scale=sc_c[:])
                        else:
                            nc.vector.tensor_scalar(
                                out=hslc, in0=xc[:, i, qcols],
                                scalar1=sc_c[:], scalar2=bi_c[:],
                                op0=ALU.mult, op1=ALU.subtract)

            # =========== Phase 2: q'/v' projections ===========
            with tc.tile_pool(name="ps_proj", bufs=3, space="PSUM") as ps_p:
                # q' = M^T h_q (queries only), stored in the interleaved fp8
                # layout (out-chunk oc -> (u=oc//2, j=oc%2)) so the scores
                # matmul runs DoubleRow. Evacuations alternate ScalarE/DVE.
                for oc in range(CC):
                    for qb in range(QB):
                        pt = ps_p.tile([128, 512], F32, name="pt", tag="pp")
                        for u in range(2):
                            nc.tensor.matmul(
                                pt[:],
                                w8_sb["w8m"][:, u, :, oc * 128:(oc + 1) * 128],
                                h8[:, u, qb, :, :],
                                start=(u == 0), stop=(u == 1), perf_mode=DR)
                        dst = q8t[:, oc // 2, qb, oc % 2, :]
                        if (oc * QB + qb) % 2 == 0:
                            nc.scalar.copy(dst, pt[:])
                        else:
                            nc.vector.tensor_copy(dst, pt[:])
                # vT'[s, c] = h[:, s]^T Wov^T  (spatial on partitions)
                for st in range(ST):
                    pt = ps_p.tile([128, 512], F32, name="pt", tag="pp")
                    ccol = slice((st % 4) * 128, (st % 4) * 128 + 128)
                    for u in range(2):
                        nc.tensor.matmul(pt[:], h8[:, u, st // 4, :, ccol],
                                         w8_sb["w8ov"][:, u, :, :],
                                         start=(u == 0), stop=(u == 1),
                                         perf_mode=DR)
                    if st % 2 == 0:
                        nc.scalar.copy(vT8[:, st, :], pt[:])
                    else:
                        nc.vector.tensor_copy(vT8[:, st, :], pt[:])

            # =========== Phase 3: attention ===========
            with tc.tile_pool(name="ps_po", bufs=4, space="PSUM") as ps_po, \
                 tc.tile_pool(name="ps_z", bufs=1, space="PSUM") as ps_z, \
                 tc.tile_pool(name="ps_s", bufs=3, space="PSUM") as ps_s:

                NP = ST // 2   # key-tile pairs (fp8 DoubleRow packs 2)

                def emit_scores_pair(qb, t):
                    e8p = work.tile([128, 2, 512], F8, name="e8p",
                                    tag="e8p", bufs=3)
                    for j in range(2):
                        st = 2 * t + j
                        pscore = ps_s.tile([128, 512], F32, name="pscore",
                                           tag="msum")
                        sc128 = slice((st % 4) * 128, (st % 4) * 128 + 128)
                        for u in range(2):
                            nc.tensor.matmul(
                                pscore[:], h8[:, u, st // 4, :, sc128],
                                q8t[:, u, qb, :, :],
                                start=(u == 0), stop=(u == 1), perf_mode=DR)
                        # e' = exp(score/sqrt(C)) * 2^-2 so fp8e4m3 never
                        # overflows; the shift cancels against Z in the
                        # final normalization
                        nc.scalar.activation(e8p[:, j, :], pscore[:], AF.Exp,
                                             scale=SCALE, bias=e8b_sb[:])
                    return e8p

                def emit_av(po, pz, t, e8p):
                    for cc2 in range(CC):
                        nc.tensor.matmul(
                            po[cc2][:],
                            vT8[:, 2 * t:2 * t + 2, cc2 * 128:(cc2 + 1) * 128],
                            e8p[:],
                            start=(t == 0), stop=(t == NP - 1), perf_mode=DR)
                    nc.tensor.matmul(pz[:], ones8[:], e8p[:],
                                     start=(t == 0), stop=(t == NP - 1),
                                     perf_mode=DR)

                for qb in range(QB):
                    po = [ps_po.tile([128, 512], F32, name="po", tag="po")
                          for _ in range(CC)]
                    pz = ps_z.tile([128, 512], F32, name="pz", tag="pz")
                    # software-pipelined: scores/exp for pair t+1 are
                    # issued before the AV matmuls of pair t, so the PE
                    # never waits on the ScalarE exp.
                    e_prev = emit_scores_pair(qb, 0)
                    for t in range(1, NP):
                        e_cur = emit_scores_pair(qb, t)
                        emit_av(po, pz, t - 1, e_prev)
                        e_prev = e_cur
                    emit_av(po, pz, NP - 1, e_prev)
                    # normalize + bias + residual + writeout. 1/Z =
                    # exp(-ln(Z)) on ScalarE (cheap, same table set); the
                    # po*rz muls and residual adds split DVE/GpSimd so the
                    # PSUM banks recycle fast and the last block's tail is
                    # short. The 2^-2 exp shift cancels between po and Z.
                    qcols = slice(qb * 512, (qb + 1) * 512)
                    zln = work.tile([128, 512], F32, name="zln", tag="zln",
                                    bufs=2)
                    nc.scalar.activation(zln[:], pz[:], AF.Ln)
                    rzb = work.tile([128, 512], F32, name="rzb", tag="rzb",
                                    bufs=2)
                    nc.scalar.activation(rzb[:], zln[:], AF.Exp, scale=-1.0)
                    for oc in range(CC):
                        # GPSIMD can't read PSUM: the po*rz mul stays on
                        # DVE; the residual add (SBUF-only) alternates onto
                        # GpSimd so the DVE tail chain stays short. Pool
                        # rejects pointer-scalar ops, so the (normally
                        # all-zero) bo bias forces the DVE path instead.
                        t32 = work.tile([128, 512], F32, name="t32",
                                        tag=f"t32_{oc % 2}", bufs=2)
                        nc.vector.tensor_mul(t32[:], po[oc][:], rzb[:])
                        o16 = work.tile([128, 512], F16, name="o16",
                                        tag=f"o16_{oc % 2}", bufs=2)
                        if with_bo:
                            nc.vector.scalar_tensor_tensor(
                                out=o16[:], in0=t32[:],
                                scalar=bo_sb[:, oc:oc + 1],
                                in1=xc[:, oc, qcols],
                                op0=ALU.add, op1=ALU.add)
                        else:
                            nc.vector.tensor_add(o16[:], t32[:],
                                                 xc[:, oc, qcols])
                        nc.sync.dma_start(out=out_d[oc, :, qcols], in_=o16[:])

    _split_excess_waits(nc)
    return nc


_cache = {}


def _get_program(with_bo=False):
    key = ("nc", with_bo)
    if key not in _cache:
        _cache[key] = _build(with_bo)
    return _cache[key]


def kernel(x, gamma, beta, wq, bq, wk, bk, wv, bv, wo, bo, trace=False):
    x = np.asarray(x, dtype=np.float32)
    gamma = np.asarray(gamma, dtype=np.float32)
    beta = np.asarray(beta, dtype=np.float32)
    wq, wk, wv, wo = (np.asarray(a, dtype=np.float32) for a in (wq, wk, wv, wo))
    bq, bk, bv, bo = (np.asarray(a, dtype=np.float32) for a in (bq, bk, bv, bo))
    assert not (np.any(bq) or np.any(bk)), \
        "nonzero bq/bk not supported by the fused-scores fast path"

    bo_eff = wo @ bv + bo
    nc = _get_program(with_bo=bool(np.any(bo_eff)))

    f8np = mybir.dt.np(F8)

    def pack8(w):
        wt = np.ascontiguousarray(w.T.astype(np.float32))
        return np.ascontiguousarray(
            wt.reshape(2, 2, 128, C).transpose(2, 0, 1, 3)).astype(f8np)

    # fold the q/k projections into M (applied to the query side only) and
    # the v/out projections into Wov; bv rides along as a constant output
    # offset (sum_s softmax = 1)
    M_T = wk.T @ wq          # device computes q' = (M_T) h_q = M^T h_q
    Wov = wo @ wv

    shared = {
        "w8m": pack8(M_T), "w8ov": pack8(Wov),
        "boc": np.ascontiguousarray(bo_eff.reshape(CC, 128).T),
        "gammac": np.ascontiguousarray(gamma.reshape(CC, 128).T),
        "betac": np.ascontiguousarray(beta.reshape(CC, 128).T),
    }
    # per-chunk group indicators: every 128-channel chunk holds the same
    # local group pattern (group = p // 16)
    ind8 = np.zeros((128, NGC), np.float32)
    ind16 = np.zeros((128, NGC), np.float16)
    indT8 = np.zeros((NGC, 128), np.float32)
    for p in range(128):
        ind8[p, p // GS] = 1.0 / (GS * S)
        ind16[p, p // GS] = 1.0   # plain 1.0 (2^-16 scale applied on-device)
        indT8[p // GS, p] = 1.0
    shared["ind8"] = ind8
    shared["ind16"] = ind16
    shared["indT8"] = indT8

    in_maps = []
    for core in range(8):
        b, half = core // 2, core % 2
        xs = x[b].reshape(C, S)
        if half:
            xin = np.concatenate([xs[:, SQ:], xs[:, :SQ]], axis=1)
        else:
            xin = np.ascontiguousarray(xs)
        in_maps.append({"x16": xin.astype(np.float16), **shared})

    res = run_bass_kernel_spmd(nc, in_maps, core_ids=list(range(8)),
                               trace=trace)
    _cache["last_exec_time_ns"] = res.exec_time_ns

    y = np.empty((B, C, S), np.float32)
    for core in range(8):
        b, half = core // 2, core % 2
        y[b, :, half * SQ:(half + 1) * SQ] = \
            res.results[core]["out"].reshape(C, SQ).astype(np.float32)
    return y.reshape(B, C, H, W)
